# revision 89
# baseline (speedup 1.0000x reference)
"""Trainium2 Bass kernel for GaussianKernelLayer.

y[n] = sum_m softmax(coef)[m] * norm * exp(-0.5*|x_n - c_m|^2),
N=500000, M=256, D=4, sigma=1. Data-parallel over 8 cores (x sharded on N).

Per core (NP=63488 padded cols = 124 chunks of 512), all four engines are
load-balanced:

  - PE: K=16 fp16 matmul computes psum[m, n] = s*(arg + F) + Bmagic, where
    arg is the exp argument, s = 1024*log2(e) (Schraudolph scale), F a window
    shift, Bmagic = 15360 + C. Centers (x2 halves of 128) stationary, x
    streams. Plus a ones-matmul partition-reduce per chunk.
  - exp is split across two engines by a static chunk schedule:
      * ACT chunks: activation Exp with scale=1/s, bias=-(Bmagic)/s - F
        undoes the scaling for free -> true exp(arg) in fp16.
      * DVE chunks: tensor_scalar max(psum,0) -> uint16 IS the exp:
        bits = s*(arg+F)+15360+C interpreted as fp16 = Schraudolph exp(arg+F).
        Host multiplies these rows by exp(-F).
  - The 256->128 m-fold runs on the DMA engines via a gpsimd-initiated
    SBUF->SBUF accumulate DMA per quarter-group (Pool pays only descriptor
    generation); the final quarter folds on DVE to keep the tail drain short.
  - A ones-matmul on PE reduces the 128 partitions per chunk (quarters of 4
    chunks share one psum bank via tile_position); ACT evacuates the reduce
    psum; DMA writes y. Reduces are deferred 8 quarters behind the mains so
    fold-DMA latency stays off the critical path.
"""

import math

import numpy as np

import concourse.bass as bass
import concourse.bacc as bacc_mod
import concourse.mybir as mybir
from concourse.bass_utils import run_bass_kernel_spmd
from concourse.tile import TileContext

N_CORES = 8
N_TOTAL = 500000
PER_CORE = N_TOTAL // N_CORES  # 62500
CHUNK = 512
NCHUNK = 124
NP = CHUNK * NCHUNK  # 63488
M = 256
D = 4
SIGMA = 1.0

F16 = mybir.dt.float16
F32 = mybir.dt.float32
U16 = mybir.dt.uint16

# Schraudolph constants (calibrated offline: rel L2 ~2.6e-3 on this data)
S_SCALE = 1024.0 * math.log2(math.e)  # 1477.3197
C_ADJ = -58.0
F_SHIFT = 10.0
B_MAGIC = 15360.0 + C_ADJ
ACT_SCALE = 1.0 / S_SCALE
ACT_BIAS = -B_MAGIC / S_SCALE - F_SHIFT

# Static chunk schedule: strictly alternate ACT / DVE exp chunks (7 ACT per
# 15) so both engines drain the shared psum window in parallel.
_PATTERN = [False, True] * 7 + [False]  # D A D A ... D, 7 A / 8 D per 15


def _act_chunks():
    return [_PATTERN[k % len(_PATTERN)] for k in range(NCHUNK)]


IS_ACT = _act_chunks()

_CACHE = {}


def _build_nc():
    nc = bacc_mod.Bacc()

    rhs_d = nc.dram_tensor("rhs", [16, NP], F16, kind="ExternalInput")
    lhsT_d = nc.dram_tensor("lhsT", [16, 256], F16, kind="ExternalInput")
    y_d = nc.dram_tensor("y", [NP], F32, kind="ExternalOutput")

    G = 16  # chunks per group (shared rhs DMA + one fold DMA)

    with TileContext(nc) as tc:
        with (
            tc.tile_pool(name="const", bufs=1) as constp,
            tc.tile_pool(name="rhsp", bufs=4) as rhsp,
            tc.tile_pool(name="expp", bufs=4) as expp,
            tc.tile_pool(name="ycp", bufs=3) as ycp,
            tc.tile_pool(name="psp", bufs=3, space="PSUM") as psp,
            tc.tile_pool(name="redp", bufs=2, space="PSUM") as redp,
        ):
            # --- constants ---
            lhsT_sb = constp.tile([16, 256], F16)
            nc.sync.dma_start(lhsT_sb[:], lhsT_d[:])
            ones_red = constp.tile([128, 32], F16)
            nc.vector.memset(ones_red[:], 1.0)
            actbias = constp.tile([128, 1], F32)
            nc.vector.memset(actbias[:], ACT_BIAS)

            def emit_quarter(qt, ex):
                """Reduce+evac+y for quarter qt (chunks 4*qt .. 4*qt+3)."""
                rp = redp.tile([128, CHUNK], F32, tag="rp", name="rp")
                for q in range(4):
                    k = 4 * qt + q
                    rcol = (k % G) * CHUNK
                    nc.tensor.matmul(
                        rp[32 * q : 32 * q + 32, :],
                        ones_red[:],
                        ex[:, 0, rcol : rcol + CHUNK],
                        start=True,
                        stop=True,
                        tile_position=(0, 32 * q),
                    )
                yc = ycp.tile([128, CHUNK], F32, tag="yc", name="yc")
                nc.scalar.copy(yc[:], rp[:])
                nc.sync.dma_start(
                    y_d[4 * qt * CHUNK : (4 * qt + 4) * CHUNK].rearrange(
                        "(p f) -> p f", p=4
                    ),
                    yc[0:97:32, :],
                )

            group_starts = list(range(0, NCHUNK, G))

            def load_rhs(g0, halves=1):
                gsz = min(G, NCHUNK - g0)
                rhs_t = rhsp.tile([16, G * CHUNK], F16, tag="rhs")
                # Tile tracks sub-ranges: splitting the first group's load
                # lets its matmuls start after the first half lands
                h = gsz * CHUNK // halves
                for s in range(halves):
                    nc.sync.dma_start(
                        rhs_t[:, s * h : (s + 1) * h],
                        rhs_d[:, g0 * CHUNK + s * h : g0 * CHUNK + (s + 1) * h],
                    )
                return rhs_t

            # prefetch three groups of rhs ahead of compute
            rhs_tiles = {g0: load_rhs(g0) for g0 in group_starts[:3]}

            quarters = []  # (quarter index, ex tile) awaiting reduce
            QDEFER = 8  # quarters of deferral
            for gi, g0 in enumerate(group_starts):
                gsz = min(G, NCHUNK - g0)
                rhs_t = rhs_tiles.pop(g0)
                if gi + 3 < len(group_starts):
                    nxt = group_starts[gi + 3]
                    rhs_tiles[nxt] = load_rhs(nxt)
                # exp outputs for the group: [p, half, G*CHUNK]
                ex = expp.tile([128, 2, G * CHUNK], F16, tag="ex")

                for kk in range(gsz):
                    k = g0 + kk
                    rcol = kk * CHUNK
                    ps = psp.tile([128, 2 * CHUNK], F32, tag="ps")
                    nc.tensor.matmul(
                        ps[:, 0:CHUNK],
                        lhsT_sb[:, 0:128],
                        rhs_t[:, rcol : rcol + CHUNK],
                        start=True,
                        stop=True,
                    )
                    nc.tensor.matmul(
                        ps[:, CHUNK : 2 * CHUNK],
                        lhsT_sb[:, 128:256],
                        rhs_t[:, rcol : rcol + CHUNK],
                        start=True,
                        stop=True,
                    )

                    ps3 = ps[:].rearrange("p (j c) -> p j c", j=2)
                    exslice = ex[:, :, rcol : rcol + CHUNK]
                    if IS_ACT[k]:
                        nc.scalar.activation(
                            exslice,
                            ps3,
                            mybir.ActivationFunctionType.Exp,
                            bias=actbias[:],
                            scale=ACT_SCALE,
                        )
                    else:
                        nc.vector.tensor_scalar(
                            exslice.bitcast(U16),
                            ps3,
                            0.0,
                            None,
                            mybir.AluOpType.max,
                        )

                    # fold each quarter-group as soon as its exps are issued
                    # so the DMA latency hides under later chunks' compute;
                    # the final quarter folds on DVE (low latency) so the
                    # tail drain isn't gated on DMA turnaround
                    if kk % 4 == 3 or kk == gsz - 1:
                        f0 = (kk // 4) * 4 * CHUNK
                        f1 = (kk + 1) * CHUNK
                        if gi == len(group_starts) - 1 and kk >= gsz - 4:
                            nc.vector.tensor_tensor(
                                ex[:, 0, f0:f1],
                                ex[:, 0, f0:f1],
                                ex[:, 1, f0:f1],
                                mybir.AluOpType.add,
                            )
                        else:
                            nc.gpsimd.dma_start(
                                ex[:, 0, f0:f1],
                                ex[:, 1, f0:f1],
                                accum_op=mybir.AluOpType.add,
                            )
                        quarters.append((k // 4, ex))
                        # interleave deferred reduces into the main stream
                        if len(quarters) > QDEFER:
                            emit_quarter(*quarters.pop(0))

            for qt, ex in quarters:
                emit_quarter(qt, ex)
    nc.compile()
    return nc


def _host_prep(x, centers, coefficients):
    """Host-side prep: softmax over 256 coefficients, Schraudolph scaling,
    fp16 hi/lo splits, per-core x layout."""
    x = np.ascontiguousarray(np.asarray(x, dtype=np.float32))
    centers = np.asarray(centers, dtype=np.float32)
    coefficients = np.asarray(coefficients, dtype=np.float32)

    norm_const = np.float32(1.0 / ((2.0 * math.pi) ** (D / 2) * SIGMA**D))
    e = np.exp(coefficients - coefficients.max())
    w = (e / e.sum()).astype(np.float32)
    b = np.log(w * norm_const) - 0.5 * (centers.astype(np.float64) ** 2).sum(axis=1)

    # center-side rows, all pre-scaled by S_SCALE
    sc = (S_SCALE * centers.T.astype(np.float64)).astype(np.float32)  # [4, 256]
    sc_hi = sc.astype(np.float16)
    sc_lo = (sc - sc_hi.astype(np.float32)).astype(np.float16)
    sb = (S_SCALE * (b + F_SHIFT) + B_MAGIC).astype(np.float32)  # [256]
    assert sb.max() + S_SCALE * 0.1 < 31000.0, "fp16-bits window overflow"
    sb_hi = sb.astype(np.float16)
    sb_lo = (sb - sb_hi.astype(np.float32)).astype(np.float16)

    lhsT = np.empty((16, 256), dtype=np.float16)
    lhsT[0:4] = sc_hi
    lhsT[4:8] = sc_lo
    lhsT[8:12] = sc_hi
    lhsT[12] = 1.0
    lhsT[13] = 1.0
    lhsT[14] = sb_hi
    lhsT[15] = sb_lo

    in_maps = []
    for i in range(N_CORES):
        xs = x[i * PER_CORE : (i + 1) * PER_CORE]
        xp = np.zeros((NP, D), dtype=np.float32)
        xp[:PER_CORE] = xs
        xh = xp.astype(np.float16)
        xl = (xp - xh.astype(np.float32)).astype(np.float16)
        h = (-0.5 * S_SCALE) * (xp.astype(np.float64) ** 2).sum(axis=1)  # [NP]
        h = h.astype(np.float32)
        h_hi = h.astype(np.float16)
        h_lo = (h - h_hi.astype(np.float32)).astype(np.float16)
        rhs = np.empty((16, NP), dtype=np.float16)
        rhs[0:4] = xh.T
        rhs[4:8] = xh.T
        rhs[8:12] = xl.T
        rhs[12] = h_hi
        rhs[13] = h_lo
        rhs[14] = 1.0
        rhs[15] = 1.0
        in_maps.append({"rhs": rhs, "lhsT": lhsT.copy()})
    return in_maps


last_result = None

# per-chunk output scale: DVE (Schraudolph) chunks carry an exp(F) factor
_CHUNK_SCALE = np.where(
    np.array(IS_ACT), 1.0, math.exp(-F_SHIFT)
).astype(np.float64)
_COL_SCALE = np.repeat(_CHUNK_SCALE, CHUNK)[:PER_CORE]  # [PER_CORE]


def kernel(x, centers, coefficients):
    global last_result
    if "nc" not in _CACHE:
        _CACHE["nc"] = _build_nc()
    nc = _CACHE["nc"]
    in_maps = _host_prep(x, centers, coefficients)
    res = run_bass_kernel_spmd(nc, in_maps, core_ids=list(range(N_CORES)))
    last_result = res
    y = np.concatenate(
        [r["y"][:PER_CORE].astype(np.float64) * _COL_SCALE for r in res.results]
    )
    return y.astype(np.float32)


# revision 90
# speedup vs baseline: 1.0076x; 1.0076x over previous
"""Trainium2 Bass kernel for GaussianKernelLayer.

y[n] = sum_m softmax(coef)[m] * norm * exp(-0.5*|x_n - c_m|^2),
N=500000, M=256, D=4, sigma=1. Data-parallel over 8 cores (x sharded on N).

Per core (NP=63488 padded cols = 124 chunks of 512), all four engines are
load-balanced:

  - PE: K=16 fp16 matmul computes psum[m, n] = s*(arg + F) + Bmagic, where
    arg is the exp argument, s = 1024*log2(e) (Schraudolph scale), F a window
    shift, Bmagic = 15360 + C. Centers (x2 halves of 128) stationary, x
    streams. Plus a ones-matmul partition-reduce per chunk.
  - exp is split across two engines by a static chunk schedule:
      * ACT chunks: activation Exp with scale=1/s, bias=-(Bmagic)/s - F
        undoes the scaling for free -> true exp(arg) in fp16.
      * DVE chunks: tensor_scalar max(psum,0) -> uint16 IS the exp:
        bits = s*(arg+F)+15360+C interpreted as fp16 = Schraudolph exp(arg+F).
        Host multiplies these rows by exp(-F).
  - The 256->128 m-fold runs on the DMA engines via a gpsimd-initiated
    SBUF->SBUF accumulate DMA per quarter-group (Pool pays only descriptor
    generation); the final quarter folds on DVE to keep the tail drain short.
  - A ones-matmul on PE reduces the 128 partitions per chunk (quarters of 4
    chunks share one psum bank via tile_position); ACT evacuates the reduce
    psum; DMA writes y. Reduces are deferred 8 quarters behind the mains so
    fold-DMA latency stays off the critical path.
"""

import math

import numpy as np

import concourse.bass as bass
import concourse.bacc as bacc_mod
import concourse.mybir as mybir
from concourse.bass_utils import run_bass_kernel_spmd
from concourse.tile import TileContext

N_CORES = 8
N_TOTAL = 500000
PER_CORE = N_TOTAL // N_CORES  # 62500
CHUNK = 512
NCHUNK = 123  # 123*512 = 62976 >= 62500: one less chunk of padding work
NP = CHUNK * NCHUNK  # 62976
M = 256
D = 4
SIGMA = 1.0

F16 = mybir.dt.float16
F32 = mybir.dt.float32
U16 = mybir.dt.uint16

# Schraudolph constants (calibrated offline: rel L2 ~2.6e-3 on this data)
S_SCALE = 1024.0 * math.log2(math.e)  # 1477.3197
C_ADJ = -58.0
F_SHIFT = 10.0
B_MAGIC = 15360.0 + C_ADJ
ACT_SCALE = 1.0 / S_SCALE
ACT_BIAS = -B_MAGIC / S_SCALE - F_SHIFT

# Static chunk schedule: strictly alternate ACT / DVE exp chunks (7 ACT per
# 15) so both engines drain the shared psum window in parallel.
_PATTERN = [False, True] * 7 + [False]  # D A D A ... D, 7 A / 8 D per 15


def _act_chunks():
    return [_PATTERN[k % len(_PATTERN)] for k in range(NCHUNK)]


IS_ACT = _act_chunks()

_CACHE = {}


def _build_nc():
    nc = bacc_mod.Bacc()

    rhs_d = nc.dram_tensor("rhs", [16, NP], F16, kind="ExternalInput")
    lhsT_d = nc.dram_tensor("lhsT", [16, 256], F16, kind="ExternalInput")
    y_d = nc.dram_tensor("y", [NP], F32, kind="ExternalOutput")

    G = 16  # chunks per group (shared rhs DMA + one fold DMA)

    with TileContext(nc) as tc:
        with (
            tc.tile_pool(name="const", bufs=1) as constp,
            tc.tile_pool(name="rhsp", bufs=4) as rhsp,
            tc.tile_pool(name="expp", bufs=4) as expp,
            tc.tile_pool(name="ycp", bufs=3) as ycp,
            tc.tile_pool(name="psp", bufs=3, space="PSUM") as psp,
            tc.tile_pool(name="redp", bufs=2, space="PSUM") as redp,
        ):
            # --- constants ---
            lhsT_sb = constp.tile([16, 256], F16)
            nc.sync.dma_start(lhsT_sb[:], lhsT_d[:])
            ones_red = constp.tile([128, 32], F16)
            nc.vector.memset(ones_red[:], 1.0)
            actbias = constp.tile([128, 1], F32)
            nc.vector.memset(actbias[:], ACT_BIAS)

            def emit_quarter(qt, ex):
                """Reduce+evac+y for quarter qt (up to 4 chunks)."""
                qsz = min(4, NCHUNK - 4 * qt)
                rp = redp.tile([128, CHUNK], F32, tag="rp", name="rp")
                for q in range(qsz):
                    k = 4 * qt + q
                    rcol = (k % G) * CHUNK
                    nc.tensor.matmul(
                        rp[32 * q : 32 * q + 32, :],
                        ones_red[:],
                        ex[:, 0, rcol : rcol + CHUNK],
                        start=True,
                        stop=True,
                        tile_position=(0, 32 * q),
                    )
                yc = ycp.tile([128, CHUNK], F32, tag="yc", name="yc")
                nc.scalar.copy(yc[:], rp[:])
                nc.sync.dma_start(
                    y_d[4 * qt * CHUNK : (4 * qt + qsz) * CHUNK].rearrange(
                        "(p f) -> p f", p=qsz
                    ),
                    yc[0 : 32 * (qsz - 1) + 1 : 32, :],
                )

            group_starts = list(range(0, NCHUNK, G))

            def load_rhs(g0, halves=1):
                gsz = min(G, NCHUNK - g0)
                rhs_t = rhsp.tile([16, G * CHUNK], F16, tag="rhs")
                # Tile tracks sub-ranges: splitting the first group's load
                # lets its matmuls start after the first half lands
                h = gsz * CHUNK // halves
                for s in range(halves):
                    nc.sync.dma_start(
                        rhs_t[:, s * h : (s + 1) * h],
                        rhs_d[:, g0 * CHUNK + s * h : g0 * CHUNK + (s + 1) * h],
                    )
                return rhs_t

            # prefetch three groups of rhs ahead of compute
            rhs_tiles = {g0: load_rhs(g0) for g0 in group_starts[:3]}

            quarters = []  # (quarter index, ex tile) awaiting reduce
            QDEFER = 8  # quarters of deferral
            for gi, g0 in enumerate(group_starts):
                gsz = min(G, NCHUNK - g0)
                rhs_t = rhs_tiles.pop(g0)
                if gi + 3 < len(group_starts):
                    nxt = group_starts[gi + 3]
                    rhs_tiles[nxt] = load_rhs(nxt)
                # exp outputs for the group: [p, half, G*CHUNK]
                ex = expp.tile([128, 2, G * CHUNK], F16, tag="ex")

                for kk in range(gsz):
                    k = g0 + kk
                    rcol = kk * CHUNK
                    ps = psp.tile([128, 2 * CHUNK], F32, tag="ps")
                    nc.tensor.matmul(
                        ps[:, 0:CHUNK],
                        lhsT_sb[:, 0:128],
                        rhs_t[:, rcol : rcol + CHUNK],
                        start=True,
                        stop=True,
                    )
                    nc.tensor.matmul(
                        ps[:, CHUNK : 2 * CHUNK],
                        lhsT_sb[:, 128:256],
                        rhs_t[:, rcol : rcol + CHUNK],
                        start=True,
                        stop=True,
                    )

                    ps3 = ps[:].rearrange("p (j c) -> p j c", j=2)
                    exslice = ex[:, :, rcol : rcol + CHUNK]
                    if IS_ACT[k]:
                        nc.scalar.activation(
                            exslice,
                            ps3,
                            mybir.ActivationFunctionType.Exp,
                            bias=actbias[:],
                            scale=ACT_SCALE,
                        )
                    else:
                        nc.vector.tensor_scalar(
                            exslice.bitcast(U16),
                            ps3,
                            0.0,
                            None,
                            mybir.AluOpType.max,
                        )

                    # fold each quarter-group as soon as its exps are issued
                    # so the DMA latency hides under later chunks' compute;
                    # the final quarter folds on DVE (low latency) so the
                    # tail drain isn't gated on DMA turnaround
                    if kk % 4 == 3 or kk == gsz - 1:
                        f0 = (kk // 4) * 4 * CHUNK
                        f1 = (kk + 1) * CHUNK
                        if gi == len(group_starts) - 1 and kk >= gsz - 4:
                            nc.vector.tensor_tensor(
                                ex[:, 0, f0:f1],
                                ex[:, 0, f0:f1],
                                ex[:, 1, f0:f1],
                                mybir.AluOpType.add,
                            )
                        else:
                            nc.gpsimd.dma_start(
                                ex[:, 0, f0:f1],
                                ex[:, 1, f0:f1],
                                accum_op=mybir.AluOpType.add,
                            )
                        quarters.append((k // 4, ex))
                        # interleave deferred reduces into the main stream
                        if len(quarters) > QDEFER:
                            emit_quarter(*quarters.pop(0))

            for qt, ex in quarters:
                emit_quarter(qt, ex)
    nc.compile()
    return nc


def _host_prep(x, centers, coefficients):
    """Host-side prep: softmax over 256 coefficients, Schraudolph scaling,
    fp16 hi/lo splits, per-core x layout."""
    x = np.ascontiguousarray(np.asarray(x, dtype=np.float32))
    centers = np.asarray(centers, dtype=np.float32)
    coefficients = np.asarray(coefficients, dtype=np.float32)

    norm_const = np.float32(1.0 / ((2.0 * math.pi) ** (D / 2) * SIGMA**D))
    e = np.exp(coefficients - coefficients.max())
    w = (e / e.sum()).astype(np.float32)
    b = np.log(w * norm_const) - 0.5 * (centers.astype(np.float64) ** 2).sum(axis=1)

    # center-side rows, all pre-scaled by S_SCALE
    sc = (S_SCALE * centers.T.astype(np.float64)).astype(np.float32)  # [4, 256]
    sc_hi = sc.astype(np.float16)
    sc_lo = (sc - sc_hi.astype(np.float32)).astype(np.float16)
    sb = (S_SCALE * (b + F_SHIFT) + B_MAGIC).astype(np.float32)  # [256]
    assert sb.max() + S_SCALE * 0.1 < 31000.0, "fp16-bits window overflow"
    sb_hi = sb.astype(np.float16)
    sb_lo = (sb - sb_hi.astype(np.float32)).astype(np.float16)

    lhsT = np.empty((16, 256), dtype=np.float16)
    lhsT[0:4] = sc_hi
    lhsT[4:8] = sc_lo
    lhsT[8:12] = sc_hi
    lhsT[12] = 1.0
    lhsT[13] = 1.0
    lhsT[14] = sb_hi
    lhsT[15] = sb_lo

    in_maps = []
    for i in range(N_CORES):
        xs = x[i * PER_CORE : (i + 1) * PER_CORE]
        xp = np.zeros((NP, D), dtype=np.float32)
        xp[:PER_CORE] = xs
        xh = xp.astype(np.float16)
        xl = (xp - xh.astype(np.float32)).astype(np.float16)
        h = (-0.5 * S_SCALE) * (xp.astype(np.float64) ** 2).sum(axis=1)  # [NP]
        h = h.astype(np.float32)
        h_hi = h.astype(np.float16)
        h_lo = (h - h_hi.astype(np.float32)).astype(np.float16)
        rhs = np.empty((16, NP), dtype=np.float16)
        rhs[0:4] = xh.T
        rhs[4:8] = xh.T
        rhs[8:12] = xl.T
        rhs[12] = h_hi
        rhs[13] = h_lo
        rhs[14] = 1.0
        rhs[15] = 1.0
        in_maps.append({"rhs": rhs, "lhsT": lhsT.copy()})
    return in_maps


last_result = None

# per-chunk output scale: DVE (Schraudolph) chunks carry an exp(F) factor
_CHUNK_SCALE = np.where(
    np.array(IS_ACT), 1.0, math.exp(-F_SHIFT)
).astype(np.float64)
_COL_SCALE = np.repeat(_CHUNK_SCALE, CHUNK)[:PER_CORE]  # [PER_CORE]


def kernel(x, centers, coefficients):
    global last_result
    if "nc" not in _CACHE:
        _CACHE["nc"] = _build_nc()
    nc = _CACHE["nc"]
    in_maps = _host_prep(x, centers, coefficients)
    res = run_bass_kernel_spmd(nc, in_maps, core_ids=list(range(N_CORES)))
    last_result = res
    y = np.concatenate(
        [r["y"][:PER_CORE].astype(np.float64) * _COL_SCALE for r in res.results]
    )
    return y.astype(np.float32)
